# revision 6
# baseline (speedup 1.0000x reference)
"""EDAC layer kernel for Trainium2 (8 NeuronCores, batch-sharded SPMD).

Reference semantics (B=32, C=256, K=64, H=W=56; vulnerable_idx == arange(K)):
  valid(x, c)  = min_vals[c] <= x <= max_vals[c]
  channels >= K:  out = x if valid else 0
  channels <  K:  m = main, d = dup
      both valid  -> min(m, d)      (covers m == d too)
      only d      -> d
      only m      -> m
      neither     -> 0

Kernel strategy (per core, 4 batches):
  rows = (batch, channel) pairs on SBUF partitions, H*W on the free dim.
  Per batch-pair (b, b+1) process five [128, HW] tiles:
    A: batch b   channels  64..191   (simple range-zero path)
    B: batch b   channels 192..255 + batch b+1 channels 64..127
    C: batch b+1 channels 128..255
    V: channels 0..63 of both batches (vulnerable, compared against dup)
    D: dup rows for both batches
  Simple path: two scalar_tensor_tensor ops  ((m>=lo)*m, then (m<=hi)*that).
  Vulnerable:  ScalarE relus r1=relu(lo-m), r2=relu(m-hi) (exact zeroness),
               m1 = m + HUGE*(r1+r2) via two stt ops, r = min(m1, d1),
               res = (r < THR) * r.
"""

import os
import sys

for _p in ("/opt/trn_rl_repo", os.path.expanduser("~/.axon_site/_ro/trn_rl_repo")):
    if os.path.isdir(_p) and _p not in sys.path:
        sys.path.insert(0, _p)

import numpy as np

import concourse.bass as bass
import concourse.bacc as bacc
import concourse.mybir as mybir
from concourse.tile import TileContext
from concourse.bass_utils import run_bass_kernel_spmd

F32 = mybir.dt.float32
BF16 = mybir.dt.bfloat16
OP = mybir.AluOpType
AF = mybir.ActivationFunctionType

B, C, K, H, W = 32, 256, 64, 56, 56
HW = H * W
NCORES = 8
BL = B // NCORES  # batches per core

HUGE = 1.0e30  # sentinel multiplier: HUGE * smallest-positive-bf16-relu >> THR
THR = 1.0e15   # valid values are <= ~10; invalid sentinels are >= ~6e22

# bounds table columns (per-partition scalars for each tile kind)
#   0..3  : lo  for tile kinds A, B, C, V
#   4..7  : hi  for tile kinds A, B, C, V
#   8..11 : -hi for tile kinds A, B, C, V
NBCOLS = 12


def build_bounds(min_vals: np.ndarray, max_vals: np.ndarray) -> np.ndarray:
    lo = np.asarray(min_vals, dtype=np.float32)
    hi = np.asarray(max_vals, dtype=np.float32)
    cols = np.zeros((128, NBCOLS), dtype=np.float32)
    kinds = [
        np.arange(64, 192),                                  # A
        np.concatenate([np.arange(192, 256), np.arange(64, 128)]),  # B
        np.arange(128, 256),                                 # C
        np.concatenate([np.arange(0, 64), np.arange(0, 64)]),       # V
    ]
    for j, idx in enumerate(kinds):
        cols[:, j] = lo[idx]
        cols[:, 4 + j] = hi[idx]
        cols[:, 8 + j] = -hi[idx]
    return cols


def build_nc(hw: int = HW) -> bass.Bass:
    nc = bacc.Bacc("TRN2", target_bir_lowering=False, debug=False)
    R = BL * C
    main = nc.dram_tensor("main", [R, hw], F32, kind="ExternalInput")
    dup = nc.dram_tensor("dup", [BL * K, hw], F32, kind="ExternalInput")
    bounds = nc.dram_tensor("bounds", [128, NBCOLS], F32, kind="ExternalInput")
    out = nc.dram_tensor("out", [R, hw], F32, kind="ExternalOutput")

    stt = nc.vector.scalar_tensor_tensor

    with TileContext(nc) as tc:
        with (
            tc.tile_pool(name="bnd", bufs=1) as bpool,
            tc.tile_pool(name="pm", bufs=5) as pm,
            tc.tile_pool(name="pd", bufs=2) as pd,
            tc.tile_pool(name="pt", bufs=5) as pt,
            tc.tile_pool(name="pr", bufs=4) as pr,
        ):
            bt = bpool.tile([128, NBCOLS], F32)
            nc.sync.dma_start(out=bt[:], in_=bounds[:])

            def lo_ap(j):
                return bt[:, j:j + 1]

            def hi_ap(j):
                return bt[:, 4 + j:5 + j]

            def nhi_ap(j):
                return bt[:, 8 + j:9 + j]

            for p in range(BL // 2):
                base = p * 2 * C      # first row of batch b=2p in main
                dbase = p * 2 * K     # first row of batch b=2p in dup

                # --- simple tiles: (kind, list of (dram_row0, nrows)) ---
                simple = [
                    (0, [(base + 64, 128)]),                        # A
                    (1, [(base + 192, 64), (base + 320, 64)]),      # B
                    (2, [(base + 384, 128)]),                       # C
                ]
                for kind, segs in simple:
                    mt = pm.tile([128, hw], F32, tag="mt")
                    row = 0
                    for r0, n in segs:
                        nc.sync.dma_start(
                            out=mt[row:row + n, :], in_=main[r0:r0 + n, :])
                        row += n
                    t1 = pt.tile([128, hw], F32, tag="t1")
                    # t1 = (m >= lo) * m
                    stt(out=t1[:], in0=mt[:], scalar=lo_ap(kind), in1=mt[:],
                        op0=OP.is_ge, op1=OP.mult)
                    # t1 = (m <= hi) * t1
                    stt(out=t1[:], in0=mt[:], scalar=hi_ap(kind), in1=t1[:],
                        op0=OP.is_le, op1=OP.mult)
                    row = 0
                    for r0, n in segs:
                        nc.sync.dma_start(
                            out=out[r0:r0 + n, :], in_=t1[row:row + n, :])
                        row += n

                # --- vulnerable tile (kind 3): channels 0..63 of both batches
                vsegs = [(base, 64), (base + C, 64)]
                mv = pm.tile([128, hw], F32, tag="mt")
                row = 0
                for r0, n in vsegs:
                    nc.sync.dma_start(out=mv[row:row + n, :], in_=main[r0:r0 + n, :])
                    row += n
                dv = pd.tile([128, hw], F32, tag="dv")
                nc.sync.dma_start(out=dv[:], in_=dup[dbase:dbase + 128, :])

                r1m = pr.tile([128, hw], BF16, tag="rl")
                r2m = pr.tile([128, hw], BF16, tag="rl")
                r1d = pr.tile([128, hw], BF16, tag="rl")
                r2d = pr.tile([128, hw], BF16, tag="rl")
                # r1 = relu(lo - x), r2 = relu(x - hi): >0 iff x invalid (exact)
                nc.scalar.activation(r1m[:], mv[:], AF.Relu, bias=lo_ap(3), scale=-1.0)
                nc.scalar.activation(r2m[:], mv[:], AF.Relu, bias=nhi_ap(3), scale=1.0)
                nc.scalar.activation(r1d[:], dv[:], AF.Relu, bias=lo_ap(3), scale=-1.0)
                nc.scalar.activation(r2d[:], dv[:], AF.Relu, bias=nhi_ap(3), scale=1.0)

                m1 = pt.tile([128, hw], F32, tag="t1")
                d1 = pt.tile([128, hw], F32, tag="t1")
                # m1 = m + HUGE*r1m + HUGE*r2m   (sentinel if m invalid)
                stt(out=m1[:], in0=r1m[:], scalar=HUGE, in1=mv[:],
                    op0=OP.mult, op1=OP.add)
                stt(out=m1[:], in0=r2m[:], scalar=HUGE, in1=m1[:],
                    op0=OP.mult, op1=OP.add)
                stt(out=d1[:], in0=r1d[:], scalar=HUGE, in1=dv[:],
                    op0=OP.mult, op1=OP.add)
                stt(out=d1[:], in0=r2d[:], scalar=HUGE, in1=d1[:],
                    op0=OP.mult, op1=OP.add)
                # m1 = min(m1, d1); res = (m1 < THR) * m1  (into d1)
                nc.vector.tensor_tensor(out=m1[:], in0=m1[:], in1=d1[:], op=OP.min)
                stt(out=d1[:], in0=m1[:], scalar=THR, in1=m1[:],
                    op0=OP.is_lt, op1=OP.mult)
                row = 0
                for r0, n in vsegs:
                    nc.sync.dma_start(out=out[r0:r0 + n, :], in_=d1[row:row + n, :])
                    row += n
    return nc


_NC_CACHE: dict = {}


def _get_nc(hw: int) -> bass.Bass:
    if hw not in _NC_CACHE:
        nc = build_nc(hw)
        nc.finalize()  # Bacc.finalize runs compile() (register allocation etc.)
        _NC_CACHE[hw] = nc
    return _NC_CACHE[hw]


def kernel(main_out, dup_out, min_vals, max_vals, vulnerable_idx):
    return _run(main_out, dup_out, min_vals, max_vals, vulnerable_idx)[0]


def _run(main_out, dup_out, min_vals, max_vals, vulnerable_idx, **spmd_kwargs):
    main_out = np.asarray(main_out)
    dup_out = np.asarray(dup_out)
    min_vals = np.asarray(min_vals)
    max_vals = np.asarray(max_vals)
    vidx = np.asarray(vulnerable_idx).ravel()

    # Device kernel assumes vulnerable channels are 0..K-1. If not, permute
    # channels host-side so they are, and invert on the way out.
    perm = None
    if not np.array_equal(vidx, np.arange(K)):
        assert len(np.unique(vidx)) == K, "duplicate vulnerable_idx unsupported"
        rest = np.setdiff1d(np.arange(C), vidx)
        perm = np.concatenate([vidx, rest])
        main_out = main_out[:, perm]
        min_vals = min_vals[perm]
        max_vals = max_vals[perm]

    mo = np.ascontiguousarray(main_out, dtype=np.float32).reshape(B, C, HW)
    du = np.ascontiguousarray(dup_out, dtype=np.float32).reshape(B, K, HW)
    bounds = build_bounds(min_vals, max_vals)

    in_maps = []
    for k in range(NCORES):
        in_maps.append({
            "main": mo[BL * k:BL * (k + 1)].reshape(BL * C, HW),
            "dup": du[BL * k:BL * (k + 1)].reshape(BL * K, HW),
            "bounds": bounds,
        })

    nc = _get_nc(HW)
    res = run_bass_kernel_spmd(nc, in_maps, list(range(NCORES)), **spmd_kwargs)
    out = np.concatenate(
        [r["out"].reshape(BL, C, H, W) for r in res.results], axis=0)

    if perm is not None:
        inv = np.empty(C, dtype=np.int64)
        inv[perm] = np.arange(C)
        out = out[:, inv]
    return out, res
